# revision 19
# baseline (speedup 1.0000x reference)
"""DCRNN encoder kernel for 8 Trainium2 NeuronCores.

Strategy
--------
Batch B=8 is sharded 1 sample per core; each core runs the full T=12 x L=2
DCGRU recurrence for its sample with zero inter-core communication.

The sparse diffusion supports are densified on the host (indices are known at
kernel-build time) and the Chebyshev recurrence  x2 = 2*A*x1 - x0  is removed
by precomputing A^2 on the host and folding the 2x / -1x coefficients into the
dense weight matrices.  Each diffusion step is then ONE PE pass that streams
the stacked dense operator  [A0^T | A0^2T | A1^T | A1^2T]  (SBUF-resident,
[1024 src, 4096 dst]) against the stationary x0 row-chunks, producing all four
diffused feature maps in transposed layout directly.

Per-core data layout is "transposed": features on partitions, the 1024 nodes
on the free dimension.  Feature order inside x0 is [h ; xi] (weight rows are
permuted on the host to match) so that h, r, and the candidate all live at
partition base 0.
"""

import numpy as np

import concourse.bass as bass
import concourse.mybir as mybir
import concourse.tile as tile
from concourse import bacc
from concourse.bass_utils import run_bass_kernel_spmd
from concourse.masks import make_identity

F32 = mybir.dt.float32
F32R = mybir.dt.float32r
import os as _os
# fp32r streams 1 col/cycle on the PE (vs 4 for fp32) at ~7.5e-5 matmul
# precision; DCRNN_MM_DT=f32 falls back to exact-fp32 matmuls.
MM_DT = F32 if _os.environ.get("DCRNN_MM_DT") == "f32" else F32R

N = 1024
H = 64
KDIFF = 2
T = 12
B = 8
IN_DIM = 2
L = 2
M = 5
NCHUNK = N // 128  # 8 src/node chunks
NOPS = 4  # A0, A0^2, A1, A1^2
F_L = [H + IN_DIM, H + H]  # feature count per layer, [h; xi] order


def _dense_support(rows, cols, vals):
    A = np.zeros((N, N), np.float64)
    np.add.at(A, (rows.astype(np.int64), cols.astype(np.int64)),
              vals.astype(np.float64))
    return A


def _fold_weights(W, F_in):
    """W: [F*M, out] with rows f*M + m, xi-features first.

    Returns [5, F, out] float32 with
      * Chebyshev fold: W0 -= W2 + W4,  W2 *= 2,  W4 *= 2
        (device computes raw powers A x0 and A^2 x0, not 2 A^2 x0 - x0)
      * feature rows permuted to [h ; xi] order.
    """
    Fin = F_in - H  # xi feature count
    Wr = W.astype(np.float64).reshape(F_in, M, -1)
    out = np.stack(
        [Wr[:, 0] - Wr[:, 2] - Wr[:, 4], Wr[:, 1], 2.0 * Wr[:, 2],
         Wr[:, 3], 2.0 * Wr[:, 4]], 0)  # [5, F, out]
    perm = np.concatenate([np.arange(Fin, F_in), np.arange(0, Fin)])
    return np.ascontiguousarray(out[:, perm]).astype(np.float32)


def _build_program(T_steps=T, layers=L):
    """Builds the per-core Bass program. Identical on all cores."""
    nc = bacc.Bacc("TRN2", target_bir_lowering=False, debug=False,
                   num_devices=B)

    # ---- DRAM parameters (per core) ----
    d_at = nc.declare_dram_parameter("at", [N, NOPS * N], MM_DT, isOutput=False)
    d_xi = nc.declare_dram_parameter("xi", [T_steps, IN_DIM, N], MM_DT,
                                     isOutput=False)
    d_h0 = nc.declare_dram_parameter("h0", [layers, H, N], MM_DT, isOutput=False)
    d_wg = [nc.declare_dram_parameter(f"wg{l}", [M, F_L[l], 2 * H], MM_DT,
                                      isOutput=False) for l in range(layers)]
    d_wc = [nc.declare_dram_parameter(f"wc{l}", [M, F_L[l], H], MM_DT,
                                      isOutput=False) for l in range(layers)]
    d_bg = [nc.declare_dram_parameter(f"bg{l}", [2 * H, 1], F32,
                                      isOutput=False) for l in range(layers)]
    d_bc = [nc.declare_dram_parameter(f"bc{l}", [H, 1], F32, isOutput=False)
            for l in range(layers)]
    d_cur = nc.declare_dram_parameter("cur", [T_steps, H, N], F32,
                                      isOutput=True)
    d_hid = nc.declare_dram_parameter("hid", [layers, H, N], F32,
                                      isOutput=True)

    with tile.TileContext(nc) as tc:
        with (
            tc.tile_pool(name="const", bufs=1) as const_pool,
            tc.tile_pool(name="state", bufs=1) as state_pool,
            tc.tile_pool(name="work", bufs=1) as work_pool,
            tc.tile_pool(name="hnew", bufs=2) as hnew_pool,
            tc.tile_pool(name="pt", bufs=2, space="PSUM") as pt_pool,
            tc.tile_pool(name="pdiff", bufs=1, space="PSUM") as pdiff_pool,
            tc.tile_pool(name="pw", bufs=1, space="PSUM") as pw_pool,
        ):
            # ---- SBUF-resident constants ----
            ident_f32 = const_pool.tile([128, 128], F32)
            make_identity(nc, ident_f32)
            if MM_DT is F32:
                ident = ident_f32
            else:
                ident = const_pool.tile([128, 128], MM_DT)
                nc.vector.tensor_copy(ident, ident_f32)

            wg_sb = [const_pool.tile([F_L[l], M, 2 * H], MM_DT, name=f"wg{l}s",
                                     tag=f"wgs{l}") for l in range(layers)]
            wc_sb = [const_pool.tile([F_L[l], M, H], MM_DT, name=f"wc{l}s",
                                     tag=f"wcs{l}") for l in range(layers)]
            bg_sb = [const_pool.tile([2 * H, 1], F32, name=f"bg{l}s",
                                     tag=f"bgs{l}") for l in range(layers)]
            bc_sb = [const_pool.tile([H, 1], F32, name=f"bc{l}s",
                                     tag=f"bcs{l}") for l in range(layers)]
            for l in range(layers):
                nc.sync.dma_start(
                    out=wg_sb[l],
                    in_=d_wg[l].ap().rearrange("m f o -> f m o"))
                nc.sync.dma_start(
                    out=wc_sb[l],
                    in_=d_wc[l].ap().rearrange("m f o -> f m o"))
                nc.sync.dma_start(out=bg_sb[l], in_=d_bg[l].ap())
                nc.sync.dma_start(out=bc_sb[l], in_=d_bc[l].ap())

            # ---- recurrent state tiles ----
            # x0_g[l]: gate-diffusion operand [h ; xi] of layer l.
            # h rows 0:H, xi rows H:F.
            x0_g = [state_pool.tile([128, N], MM_DT, name=f"x0g{l}",
                                    tag=f"x0g{l}") for l in range(layers)]
            for l in range(layers):
                nc.sync.dma_start(out=x0_g[l][0:H, :], in_=d_h0.ap()[l])
            nc.sync.dma_start(out=x0_g[0][H:F_L[0], :], in_=d_xi.ap()[0])

            at_oc = [[const_pool.tile([128, N], MM_DT,
                                       name=f"at{j}_{c}", tag=f"at{j}_{c}")
                      for c in range(NCHUNK)] for j in range(NOPS)]
            at_view = d_at.ap().rearrange("(c p) (j n) -> p c j n",
                                          p=128, j=NOPS)
            for c in range(NCHUNK):
                for j in range(NOPS):
                    nc.sync.dma_start(out=at_oc[j][c],
                                      in_=at_view[:, c, j, :])


            def diffuse_wmm(x0T, F, w_sb_l, out_dim, xi_from=None):
                """Diffusion + fused weight matmul.

                x0T [128, N] (rows 0:F valid) -> psum [out_dim, N] =
                sum_m W_m^T @ mats_m^T with mats = [x0, A0 x, A0^2 x,
                A1 x, A1^2 x] (raw powers; chebyshev is folded in W).
                If xi_from is given (prior x0row with identical xi rows),
                only the h rows are transposed; xi columns are copied.
                """
                x0row = work_pool.tile([128, NCHUNK, 128], MM_DT, tag="x0row",
                                       bufs=2)
                tr_rows = H if xi_from is not None else F
                for c in range(NCHUNK):
                    pt = pt_pool.tile([128, 128], MM_DT, tag="pt")
                    nc.tensor.transpose(pt[:, 0:tr_rows],
                                        x0T[0:tr_rows,
                                            c * 128:(c + 1) * 128],
                                        ident[0:tr_rows, 0:tr_rows])
                    nc.scalar.copy(x0row[:, c, 0:tr_rows], pt[:, 0:tr_rows])
                    if xi_from is not None and F > H:
                        nc.vector.tensor_copy(x0row[:, c, H:F],
                                              xi_from[:, c, H:F])
                pw = pw_pool.tile([128, N], F32, tag="pw")
                for half in range(2):
                    pd = pdiff_pool.tile([128, NOPS * 512], F32, tag="pd")
                    for c in range(NCHUNK):
                        for j in range(NOPS):
                            nc.tensor.matmul(
                                pd[0:F, j * 512:(j + 1) * 512],
                                x0row[:, c, 0:F],
                                at_oc[j][c][:, half * 512:half * 512 + 512],
                                start=(c == 0), stop=(c == NCHUNK - 1))
                    mh = work_pool.tile([128, NOPS * 512], MM_DT, tag="matsh",
                                        bufs=2)
                    for j in range(NOPS):
                        nc.vector.tensor_copy(
                            mh[0:F, j * 512:(j + 1) * 512],
                            pd[0:F, j * 512:(j + 1) * 512])
                    for m in range(M):
                        if m == 0:
                            rhs = x0T[0:F, half * 512:half * 512 + 512]
                        else:
                            rhs = mh[0:F, (m - 1) * 512:m * 512]
                        nc.tensor.matmul(
                            pw[0:out_dim, half * 512:half * 512 + 512],
                            w_sb_l[0:F, m, 0:out_dim], rhs,
                            start=(m == 0), stop=(m == M - 1))
                return pw, x0row

            for t in range(T_steps):
                for l in range(layers):
                    F = F_L[l]
                    xg = x0_g[l]
                    if l == 0 and t > 0:
                        nc.sync.dma_start(out=xg[H:F, :], in_=d_xi.ap()[t])

                    # ---- gates path ----
                    pg, xrow_g = diffuse_wmm(xg, F, wg_sb[l], 2 * H)
                    gates = work_pool.tile([128, N], F32, tag="gates")
                    for hh in range(4):
                        cs = slice(hh * 256, hh * 256 + 256)
                        nc.scalar.activation(
                            gates[:, cs], pg[:, cs],
                            mybir.ActivationFunctionType.Sigmoid,
                            bias=bg_sb[l])
                    # u lives at partitions 64:128; engines need matching
                    # partition bases, so realign via DMA (overlaps cand path)
                    u_al = work_pool.tile([H, N], F32, tag="ual")
                    nc.sync.dma_start(out=u_al, in_=gates[H:2 * H, :])
                    # off-critical-path: 1-u and u*h (overlap cand diffusion)
                    uc = work_pool.tile([H, N], F32, tag="uc")
                    nc.scalar.activation(uc, u_al,
                                         mybir.ActivationFunctionType.Identity,
                                         bias=1.0, scale=-1.0)
                    t1 = work_pool.tile([H, N], F32, tag="t1")
                    nc.vector.tensor_mul(t1, u_al, xg[0:H, :])

                    # ---- candidate path ----
                    xc = work_pool.tile([128, N], MM_DT, tag="x0c")
                    for hh in range(4):
                        cs = slice(hh * 256, hh * 256 + 256)
                        nc.vector.tensor_mul(xc[0:H, cs], gates[0:H, cs],
                                             xg[0:H, cs])
                    if F > H:
                        nc.vector.tensor_copy(xc[H:F, :], xg[H:F, :])
                    pc, _ = diffuse_wmm(xc, F, wc_sb[l], H, xi_from=xrow_g)
                    cand = work_pool.tile([H, N], F32, tag="cand")
                    for hh in range(4):
                        cs = slice(hh * 256, hh * 256 + 256)
                        nc.scalar.activation(cand[:, cs], pc[0:H, cs],
                                             mybir.ActivationFunctionType.Tanh,
                                             bias=bc_sb[l])

                    # ---- state update: h_new = u*h + (1-u)*c ----
                    for hh in range(4):
                        cs = slice(hh * 256, hh * 256 + 256)
                        nc.vector.tensor_mul(cand[:, cs], uc[:, cs],
                                             cand[:, cs])
                    if l + 1 < layers:
                        # write h_new straight into layer l+1's xi rows; the
                        # layer-l state tile is refreshed lazily (off the
                        # critical path) during layer l+1's cell.
                        dest = x0_g[l + 1][H:2 * H, :]
                        for hh in range(4):
                            cs = slice(hh * 256, hh * 256 + 256)
                            nc.vector.tensor_add(dest[:, cs], t1[:, cs],
                                                 cand[:, cs])
                        nc.scalar.copy(x0_g[l][0:H, :], dest)
                        if t == T_steps - 1:
                            nc.sync.dma_start(out=d_hid.ap()[l], in_=dest.bitcast(F32))
                    else:
                        dest = x0_g[l][0:H, :]
                        for hh in range(4):
                            cs = slice(hh * 256, hh * 256 + 256)
                            nc.vector.tensor_add(dest[:, cs], t1[:, cs],
                                                 cand[:, cs])
                        nc.sync.dma_start(out=d_cur.ap()[t], in_=dest.bitcast(F32))
                        if t == T_steps - 1:
                            nc.sync.dma_start(out=d_hid.ap()[l], in_=dest.bitcast(F32))

    nc.compile()
    return nc


_CACHE = {}


def _get_program():
    if "nc" not in _CACHE:
        _CACHE["nc"] = _build_program()
    return _CACHE["nc"]


def _host_inputs(inputs, initial_hidden_state, support_rows, support_cols,
                 support_vals, Wg0, bg0, Wc0, bc0, Wg1, bg1, Wc1, bc1,
                 T_steps=T, layers=L):
    A0 = _dense_support(support_rows[0], support_cols[0], support_vals[0])
    A1 = _dense_support(support_rows[1], support_cols[1], support_vals[1])
    at = np.concatenate(
        [A0.T, (A0 @ A0).T, A1.T, (A1 @ A1).T], axis=1).astype(np.float32)
    at = np.ascontiguousarray(at)

    wg = [_fold_weights(Wg0, F_L[0]), _fold_weights(Wg1, F_L[1])]
    wc = [_fold_weights(Wc0, F_L[0]), _fold_weights(Wc1, F_L[1])]
    bg = [np.ascontiguousarray(bg0.astype(np.float32).reshape(2 * H, 1)),
          np.ascontiguousarray(bg1.astype(np.float32).reshape(2 * H, 1))]
    bc = [np.ascontiguousarray(bc0.astype(np.float32).reshape(H, 1)),
          np.ascontiguousarray(bc1.astype(np.float32).reshape(H, 1))]

    in_maps = []
    for b in range(B):
        # xi: [T, B, N*IN_DIM] -> per core [T, IN_DIM, N] transposed
        xi = np.ascontiguousarray(
            inputs[:T_steps, b].reshape(T_steps, N, IN_DIM)
            .transpose(0, 2, 1)).astype(np.float32)
        h0 = np.ascontiguousarray(
            initial_hidden_state[:layers, b].reshape(layers, N, H)
            .transpose(0, 2, 1)).astype(np.float32)
        m = {"at": at, "xi": xi, "h0": h0}
        for l in range(layers):
            m[f"wg{l}"] = wg[l]
            m[f"wc{l}"] = wc[l]
            m[f"bg{l}"] = bg[l]
            m[f"bc{l}"] = bc[l]
        in_maps.append(m)
    return in_maps


def kernel(inputs, initial_hidden_state, support_rows, support_cols,
           support_vals, Wg0, bg0, Wc0, bc0, Wg1, bg1, Wc1, bc1):
    inputs = np.asarray(inputs, np.float32)
    initial_hidden_state = np.asarray(initial_hidden_state, np.float32)
    in_maps = _host_inputs(
        inputs, initial_hidden_state,
        np.asarray(support_rows), np.asarray(support_cols),
        np.asarray(support_vals, np.float32),
        np.asarray(Wg0, np.float32), np.asarray(bg0, np.float32),
        np.asarray(Wc0, np.float32), np.asarray(bc0, np.float32),
        np.asarray(Wg1, np.float32), np.asarray(bg1, np.float32),
        np.asarray(Wc1, np.float32), np.asarray(bc1, np.float32))

    nc = _get_program()
    import os
    tmpdir = os.environ.get("DCRNN_TRACE_DIR") or None
    res = run_bass_kernel_spmd(nc, in_maps, list(range(B)), tmpdir=tmpdir)
    _CACHE["last_results"] = res

    hid = np.zeros((L, B, N * H), np.float32)
    cur = np.zeros((T, B, N * H), np.float32)
    for b in range(B):
        r = res.results[b]
        hid[:, b] = r["hid"].transpose(0, 2, 1).reshape(L, N * H)
        cur[:, b] = r["cur"].transpose(0, 2, 1).reshape(T, N * H)
    return hid, cur


# revision 21
# speedup vs baseline: 1.0443x; 1.0443x over previous
"""DCRNN encoder kernel for 8 Trainium2 NeuronCores.

Strategy
--------
Batch B=8 is sharded 1 sample per core; each core runs the full T=12 x L=2
DCGRU recurrence for its sample with zero inter-core communication.

The sparse diffusion supports are densified on the host (indices are known at
kernel-build time) and the Chebyshev recurrence  x2 = 2*A*x1 - x0  is removed
by precomputing A^2 on the host and folding the 2x / -1x coefficients into the
dense weight matrices.  Each diffusion step is then ONE PE pass that streams
the stacked dense operator  [A0^T | A0^2T | A1^T | A1^2T]  (SBUF-resident,
[1024 src, 4096 dst]) against the stationary x0 row-chunks, producing all four
diffused feature maps in transposed layout directly.

Per-core data layout is "transposed": features on partitions, the 1024 nodes
on the free dimension.  Feature order inside x0 is [h ; xi] (weight rows are
permuted on the host to match) so that h, r, and the candidate all live at
partition base 0.
"""

import numpy as np

import concourse.bass as bass
import concourse.mybir as mybir
import concourse.tile as tile
from concourse import bacc
from concourse.bass_utils import run_bass_kernel_spmd
from concourse.masks import make_identity

F32 = mybir.dt.float32
F32R = mybir.dt.float32r
import os as _os
# fp32r streams 1 col/cycle on the PE (vs 4 for fp32) at ~7.5e-5 matmul
# precision; DCRNN_MM_DT=f32 falls back to exact-fp32 matmuls.
MM_DT = F32 if _os.environ.get("DCRNN_MM_DT") == "f32" else F32R

N = 1024
H = 64
KDIFF = 2
T = 12
B = 8
IN_DIM = 2
L = 2
M = 5
NCHUNK = N // 128  # 8 src/node chunks
NOPS = 4  # A0, A0^2, A1, A1^2
F_L = [H + IN_DIM, H + H]  # feature count per layer, [h; xi] order


def _dense_support(rows, cols, vals):
    A = np.zeros((N, N), np.float64)
    np.add.at(A, (rows.astype(np.int64), cols.astype(np.int64)),
              vals.astype(np.float64))
    return A


def _fold_weights(W, F_in):
    """W: [F*M, out] with rows f*M + m, xi-features first.

    Returns [5, F, out] float32 with
      * Chebyshev fold: W0 -= W2 + W4,  W2 *= 2,  W4 *= 2
        (device computes raw powers A x0 and A^2 x0, not 2 A^2 x0 - x0)
      * feature rows permuted to [h ; xi] order.
    """
    Fin = F_in - H  # xi feature count
    Wr = W.astype(np.float64).reshape(F_in, M, -1)
    out = np.stack(
        [Wr[:, 0] - Wr[:, 2] - Wr[:, 4], Wr[:, 1], 2.0 * Wr[:, 2],
         Wr[:, 3], 2.0 * Wr[:, 4]], 0)  # [5, F, out]
    perm = np.concatenate([np.arange(Fin, F_in), np.arange(0, Fin)])
    return np.ascontiguousarray(out[:, perm]).astype(np.float32)


def _build_program(T_steps=T, layers=L):
    """Builds the per-core Bass program. Identical on all cores."""
    nc = bacc.Bacc("TRN2", target_bir_lowering=False, debug=False,
                   num_devices=B)

    # ---- DRAM parameters (per core) ----
    d_at = nc.declare_dram_parameter("at", [N, NOPS * N], MM_DT, isOutput=False)
    d_xi = nc.declare_dram_parameter("xi", [T_steps, IN_DIM, N], MM_DT,
                                     isOutput=False)
    d_h0 = nc.declare_dram_parameter("h0", [layers, H, N], MM_DT, isOutput=False)
    d_wg = [nc.declare_dram_parameter(f"wg{l}", [M, F_L[l], 2 * H], MM_DT,
                                      isOutput=False) for l in range(layers)]
    d_wc = [nc.declare_dram_parameter(f"wc{l}", [M, F_L[l], H], MM_DT,
                                      isOutput=False) for l in range(layers)]
    d_bg = [nc.declare_dram_parameter(f"bg{l}", [2 * H, 1], F32,
                                      isOutput=False) for l in range(layers)]
    d_bc = [nc.declare_dram_parameter(f"bc{l}", [H, 1], F32, isOutput=False)
            for l in range(layers)]
    d_cur = nc.declare_dram_parameter("cur", [T_steps, H, N], F32,
                                      isOutput=True)
    d_hid = nc.declare_dram_parameter("hid", [layers, H, N], F32,
                                      isOutput=True)

    with tile.TileContext(nc) as tc:
        with (
            tc.tile_pool(name="const", bufs=1) as const_pool,
            tc.tile_pool(name="state", bufs=1) as state_pool,
            tc.tile_pool(name="work", bufs=1) as work_pool,
            tc.tile_pool(name="hnew", bufs=2) as hnew_pool,
            tc.tile_pool(name="pt", bufs=2, space="PSUM") as pt_pool,
            tc.tile_pool(name="pdiff", bufs=1, space="PSUM") as pdiff_pool,
            tc.tile_pool(name="pw", bufs=1, space="PSUM") as pw_pool,
        ):
            # ---- SBUF-resident constants ----
            ident_f32 = const_pool.tile([128, 128], F32)
            make_identity(nc, ident_f32)
            if MM_DT is F32:
                ident = ident_f32
            else:
                ident = const_pool.tile([128, 128], MM_DT)
                nc.vector.tensor_copy(ident, ident_f32)

            wg_sb = [const_pool.tile([F_L[l], M, 2 * H], MM_DT, name=f"wg{l}s",
                                     tag=f"wgs{l}") for l in range(layers)]
            wc_sb = [const_pool.tile([F_L[l], M, H], MM_DT, name=f"wc{l}s",
                                     tag=f"wcs{l}") for l in range(layers)]
            bg_sb = [const_pool.tile([2 * H, 1], F32, name=f"bg{l}s",
                                     tag=f"bgs{l}") for l in range(layers)]
            bc_sb = [const_pool.tile([H, 1], F32, name=f"bc{l}s",
                                     tag=f"bcs{l}") for l in range(layers)]
            for l in range(layers):
                nc.sync.dma_start(
                    out=wg_sb[l],
                    in_=d_wg[l].ap().rearrange("m f o -> f m o"))
                nc.sync.dma_start(
                    out=wc_sb[l],
                    in_=d_wc[l].ap().rearrange("m f o -> f m o"))
                nc.sync.dma_start(out=bg_sb[l], in_=d_bg[l].ap())
                nc.sync.dma_start(out=bc_sb[l], in_=d_bc[l].ap())

            # ---- recurrent state tiles ----
            # x0_g[l]: gate-diffusion operand [h ; xi] of layer l.
            # h rows 0:H, xi rows H:F.
            x0_g = [state_pool.tile([128, N], MM_DT, name=f"x0g{l}",
                                    tag=f"x0g{l}") for l in range(layers)]
            for l in range(layers):
                nc.sync.dma_start(out=x0_g[l][0:H, :], in_=d_h0.ap()[l])
            nc.sync.dma_start(out=x0_g[0][H:F_L[0], :], in_=d_xi.ap()[0])

            at_oc = [[const_pool.tile([128, N], MM_DT,
                                       name=f"at{j}_{c}", tag=f"at{j}_{c}")
                      for c in range(NCHUNK)] for j in range(NOPS)]
            at_view = d_at.ap().rearrange("(c p) (j n) -> p c j n",
                                          p=128, j=NOPS)
            for c in range(NCHUNK):
                for j in range(NOPS):
                    nc.sync.dma_start(out=at_oc[j][c],
                                      in_=at_view[:, c, j, :])


            def diffuse_wmm(x0T, F, w_sb_l, out_dim, xi_from=None):
                """Diffusion + fused weight matmul.

                x0T [128, N] (rows 0:F valid) -> psum [out_dim, N] =
                sum_m W_m^T @ mats_m^T with mats = [x0, A0 x, A0^2 x,
                A1 x, A1^2 x] (raw powers; chebyshev is folded in W).
                If xi_from is given (prior x0row with identical xi rows),
                only the h rows are transposed; xi columns are copied.
                """
                x0row = work_pool.tile([128, NCHUNK, 128], MM_DT, tag="x0row",
                                       bufs=3)
                for c in range(NCHUNK):
                    pt = pt_pool.tile([128, 128], MM_DT, tag="pt")
                    nc.tensor.transpose(pt, x0T[:, c * 128:(c + 1) * 128],
                                        ident)
                    nc.scalar.copy(x0row[:, c, 0:F], pt[:, 0:F])
                pw = pw_pool.tile([128, N], F32, tag="pw")
                for half in range(2):
                    pd = pdiff_pool.tile([128, NOPS * 512], F32, tag="pd")
                    for c in range(NCHUNK):
                        for j in range(NOPS):
                            nc.tensor.matmul(
                                pd[0:F, j * 512:(j + 1) * 512],
                                x0row[:, c, 0:F],
                                at_oc[j][c][:, half * 512:half * 512 + 512],
                                start=(c == 0), stop=(c == NCHUNK - 1))
                    mh = work_pool.tile([128, NOPS * 512], MM_DT, tag="matsh",
                                        bufs=3)
                    for j in range(NOPS):
                        eng = nc.vector.tensor_copy if j % 2 == 0 \
                            else nc.scalar.copy
                        eng(mh[0:F, j * 512:(j + 1) * 512],
                            pd[0:F, j * 512:(j + 1) * 512])
                    for m in range(M):
                        if m == 0:
                            rhs = x0T[0:F, half * 512:half * 512 + 512]
                        else:
                            rhs = mh[0:F, (m - 1) * 512:m * 512]
                        nc.tensor.matmul(
                            pw[0:out_dim, half * 512:half * 512 + 512],
                            w_sb_l[0:F, m, 0:out_dim], rhs,
                            start=(m == 0), stop=(m == M - 1))
                return pw, x0row

            for t in range(T_steps):
                for l in range(layers):
                    F = F_L[l]
                    xg = x0_g[l]
                    if l == 0 and t > 0:
                        nc.sync.dma_start(out=xg[H:F, :], in_=d_xi.ap()[t])

                    # ---- gates path ----
                    pg, xrow_g = diffuse_wmm(xg, F, wg_sb[l], 2 * H)
                    gates = work_pool.tile([128, N], F32, tag="gates")
                    for hh in range(4):
                        cs = slice(hh * 256, hh * 256 + 256)
                        nc.scalar.activation(
                            gates[:, cs], pg[:, cs],
                            mybir.ActivationFunctionType.Sigmoid,
                            bias=bg_sb[l])
                    # u lives at partitions 64:128; engines need matching
                    # partition bases, so realign via DMA (overlaps cand path)
                    u_al = work_pool.tile([H, N], F32, tag="ual")
                    nc.sync.dma_start(out=u_al, in_=gates[H:2 * H, :])
                    # off-critical-path: 1-u and u*h (overlap cand diffusion)
                    uc = work_pool.tile([H, N], F32, tag="uc")
                    nc.scalar.activation(uc, u_al,
                                         mybir.ActivationFunctionType.Identity,
                                         bias=1.0, scale=-1.0)
                    t1 = work_pool.tile([H, N], F32, tag="t1")
                    nc.vector.tensor_mul(t1, u_al, xg[0:H, :])

                    # ---- candidate path ----
                    xc = work_pool.tile([128, N], MM_DT, tag="x0c")
                    for hh in range(4):
                        cs = slice(hh * 256, hh * 256 + 256)
                        nc.vector.tensor_mul(xc[0:H, cs], gates[0:H, cs],
                                             xg[0:H, cs])
                    if F > H:
                        nc.vector.tensor_copy(xc[H:F, :], xg[H:F, :])
                    pc, _ = diffuse_wmm(xc, F, wc_sb[l], H, xi_from=xrow_g)
                    cand = work_pool.tile([H, N], F32, tag="cand")
                    for hh in range(4):
                        cs = slice(hh * 256, hh * 256 + 256)
                        nc.scalar.activation(cand[:, cs], pc[0:H, cs],
                                             mybir.ActivationFunctionType.Tanh,
                                             bias=bc_sb[l])

                    # ---- state update: h_new = u*h + (1-u)*c ----
                    for hh in range(4):
                        cs = slice(hh * 256, hh * 256 + 256)
                        nc.vector.tensor_mul(cand[:, cs], uc[:, cs],
                                             cand[:, cs])
                    if l + 1 < layers:
                        # write h_new straight into layer l+1's xi rows; the
                        # layer-l state tile is refreshed lazily (off the
                        # critical path) during layer l+1's cell.
                        dest = x0_g[l + 1][H:2 * H, :]
                        for hh in range(4):
                            cs = slice(hh * 256, hh * 256 + 256)
                            nc.vector.tensor_add(dest[:, cs], t1[:, cs],
                                                 cand[:, cs])
                        nc.scalar.copy(x0_g[l][0:H, :], dest)
                        if t == T_steps - 1:
                            nc.sync.dma_start(out=d_hid.ap()[l], in_=dest.bitcast(F32))
                    else:
                        dest = x0_g[l][0:H, :]
                        for hh in range(4):
                            cs = slice(hh * 256, hh * 256 + 256)
                            nc.vector.tensor_add(dest[:, cs], t1[:, cs],
                                                 cand[:, cs])
                        nc.sync.dma_start(out=d_cur.ap()[t], in_=dest.bitcast(F32))
                        if t == T_steps - 1:
                            nc.sync.dma_start(out=d_hid.ap()[l], in_=dest.bitcast(F32))

    nc.compile()
    return nc


_CACHE = {}


def _get_program():
    if "nc" not in _CACHE:
        _CACHE["nc"] = _build_program()
    return _CACHE["nc"]


def _host_inputs(inputs, initial_hidden_state, support_rows, support_cols,
                 support_vals, Wg0, bg0, Wc0, bc0, Wg1, bg1, Wc1, bc1,
                 T_steps=T, layers=L):
    A0 = _dense_support(support_rows[0], support_cols[0], support_vals[0])
    A1 = _dense_support(support_rows[1], support_cols[1], support_vals[1])
    at = np.concatenate(
        [A0.T, (A0 @ A0).T, A1.T, (A1 @ A1).T], axis=1).astype(np.float32)
    at = np.ascontiguousarray(at)

    wg = [_fold_weights(Wg0, F_L[0]), _fold_weights(Wg1, F_L[1])]
    wc = [_fold_weights(Wc0, F_L[0]), _fold_weights(Wc1, F_L[1])]
    bg = [np.ascontiguousarray(bg0.astype(np.float32).reshape(2 * H, 1)),
          np.ascontiguousarray(bg1.astype(np.float32).reshape(2 * H, 1))]
    bc = [np.ascontiguousarray(bc0.astype(np.float32).reshape(H, 1)),
          np.ascontiguousarray(bc1.astype(np.float32).reshape(H, 1))]

    in_maps = []
    for b in range(B):
        # xi: [T, B, N*IN_DIM] -> per core [T, IN_DIM, N] transposed
        xi = np.ascontiguousarray(
            inputs[:T_steps, b].reshape(T_steps, N, IN_DIM)
            .transpose(0, 2, 1)).astype(np.float32)
        h0 = np.ascontiguousarray(
            initial_hidden_state[:layers, b].reshape(layers, N, H)
            .transpose(0, 2, 1)).astype(np.float32)
        m = {"at": at, "xi": xi, "h0": h0}
        for l in range(layers):
            m[f"wg{l}"] = wg[l]
            m[f"wc{l}"] = wc[l]
            m[f"bg{l}"] = bg[l]
            m[f"bc{l}"] = bc[l]
        in_maps.append(m)
    return in_maps


def kernel(inputs, initial_hidden_state, support_rows, support_cols,
           support_vals, Wg0, bg0, Wc0, bc0, Wg1, bg1, Wc1, bc1):
    inputs = np.asarray(inputs, np.float32)
    initial_hidden_state = np.asarray(initial_hidden_state, np.float32)
    in_maps = _host_inputs(
        inputs, initial_hidden_state,
        np.asarray(support_rows), np.asarray(support_cols),
        np.asarray(support_vals, np.float32),
        np.asarray(Wg0, np.float32), np.asarray(bg0, np.float32),
        np.asarray(Wc0, np.float32), np.asarray(bc0, np.float32),
        np.asarray(Wg1, np.float32), np.asarray(bg1, np.float32),
        np.asarray(Wc1, np.float32), np.asarray(bc1, np.float32))

    nc = _get_program()
    import os
    tmpdir = os.environ.get("DCRNN_TRACE_DIR") or None
    res = run_bass_kernel_spmd(nc, in_maps, list(range(B)), tmpdir=tmpdir)
    _CACHE["last_results"] = res

    hid = np.zeros((L, B, N * H), np.float32)
    cur = np.zeros((T, B, N * H), np.float32)
    for b in range(B):
        r = res.results[b]
        hid[:, b] = r["hid"].transpose(0, 2, 1).reshape(L, N * H)
        cur[:, b] = r["cur"].transpose(0, 2, 1).reshape(T, N * H)
    return hid, cur
